# revision 4
# baseline (speedup 1.0000x reference)
"""Trainium2 Bass kernel for nn_AttnBlock (sparse 3x3-window attention).

Restructuring (~1.9x vs the previous kernel, TimelineSim ~220us/core):
  - Scores: banded q.k^T matmuls on the PE per a-tile of 128 pixels (band of
    268 absolute positions, per-head partition-subrange contraction), instead
    of 150 DVE product ops + PE selection-matmul reduction.
  - exp() on the whole band on ACT; the 25 displacement maps F_e[a] are then
    extracted with a skewed DMA read through a DRAM roundtrip (diagonal access
    patterns are expressible on flat DRAM, not on SBUF).
  - Softmax denominators: 3x3 box sums along the displacement axis (DVE+Pool);
    P = F * 1/G; window column-sums W[n,(j,h)] assembled with constant
    shift-diagonal matmuls (one wide diagonal "shiftbank" constant).
  - o_mean: W is scattered into a zero-filled DRAM band image (h-innermost so
    the scatter has 16B runs), read back as skewed [a, n, h] tiles, and o =
    vT^T @ Wband runs as banded PE matmuls against a transposed v (produced
    directly by stationary-swapped qkv matmuls).  proj is pipelined per n-tile.
  - LN2 is algebraically folded into a transposed fc: y^T = Relu(rstd[n] *
    (u^T @ (wfc*g2) + negmu[n]*colsum + sqrtvar[n]*bias_row)), with rstd as a
    per-partition ACT scale; the y tiles are PE-transposed back to [o, n].

Sharding: data-parallel over batch B=8 -> one batch per NeuronCore.
"""

import functools
import numpy as np
import ml_dtypes

import concourse.bass as bass
import concourse.mybir as mybir
import concourse.tile as tile
from concourse import bacc
from concourse.bass_utils import run_bass_kernel_spmd

F32 = mybir.dt.float32
BF16 = mybir.dt.bfloat16
AF = mybir.ActivationFunctionType
ALU = mybir.AluOpType
AP = bass.AP

C = 768
NCH = 6          # channel chunks of 128
G = 34           # padded grid side
A = G * G        # 1156 padded pixels
AW = 1160        # padded-pixel width with 4 pad cols
NW = 1088        # window-grid width = 32*34 (rows 0..31, cols 0..33)
KW = 1300        # k map width with +-70 margins (content at 70)
HEADS = 8
HD = 96
SCALE = HD ** -0.5
EPS = 1e-5

# segments over the a-grid (1156) and n-grid (1088); PSUM tile is [P, 3, 512]
SEG_A = [(0, 386), (386, 386), (772, 384)]
SEG_N = [(0, 384), (384, 384), (768, 320)]

KI_LIST = [(r, c) for r in range(3) for c in range(3)]             # 9
S_LIST = [34 * r + c for (r, c) in KI_LIST]                        # window offsets

# a-tiles and n-tiles of 128
AT = [(128 * t, 128) for t in range(9)] + [(1152, 4)]              # 10 tiles
NT = [(128 * t, 128) for t in range(8)] + [(1024, 64)]             # 9 tiles

# head h -> list of (chunk, p0, p1) pieces covering d-range [96h, 96h+96).
# PE tile_position rules: size<=32 -> base in {0,32,64,96}; size<=64 -> {0,64};
# else base 0.  Split pieces starting at 32 so each is legal.
def _head_pieces(h):
    lo, hi = 96 * h, 96 * h + 96
    out = []
    g0, g1 = lo // 128, (hi - 1) // 128
    for g in range(g0, g1 + 1):
        p0 = max(lo - 128 * g, 0)
        p1 = min(hi - 128 * g, 128)
        if p0 == 32 and p1 > 64:
            out.append((g, 32, 64))
            out.append((g, 64, p1))
        else:
            out.append((g, p0, p1))
    return out

HEAD_PIECES = [_head_pieces(h) for h in range(HEADS)]
# head groups per psum tile: 3 + 3 + 2
HGROUPS = [(0, 3), (3, 3), (6, 2)]
FDW = 2144       # dram band pitch: 8 heads x 268

# W-band image, h-interleaved: cell (a, d', h) at flat (a*268 + d')*8 + h;
# content = W[n = a - 70 + d', j: s_j = 70 - d', h] for d' in {70 - s}, else 0
IMR = 268                # image row pitch (in cells)
IMT = 1160 * IMR * HEADS


# c-chunk -> list of (p0, p1, h) out-partition segments with legal tile pos
def _chunk_segs(cch):
    lo = 128 * cch
    bounds = sorted({lo, lo + 128} |
                    {96 * h for h in range(1, 8) if lo < 96 * h < lo + 128})
    segs = []
    for b0, b1 in zip(bounds[:-1], bounds[1:]):
        p0, p1 = b0 - lo, b1 - lo
        h = b0 // 96
        if p0 == 32 and p1 - p0 > 32:
            segs.append((32, 64, h))
            segs.append((64, p1, h))
        else:
            segs.append((p0, p1, h))
    return segs


CHUNK_SEGS = [_chunk_segs(c) for c in range(NCH)]


def _ragged(ap_flat, segs):
    return [ap_flat[:, o:o + w] for (o, w) in segs]


def _ps_ragged(ps, segs):
    return [ps[:, s, 0:w] for s, (o, w) in enumerate(segs)]


def emit_kernel(ctx, tc, ins, outs):
    nc = tc.nc
    xp_d = ins["xp"]          # [6,128,1156] bf16
    wq_d = ins["wqkv"]        # [6,128,2304] bf16
    wp_d = ins["wproj"]       # [6,128,768] bf16
    wf_d = ins["wfc"]         # [6,128,256] bf16
    bqkv_d = ins["bqkv"]      # [128,18] f32
    bproj_d = ins["bproj"]    # [128,6] f32
    bfc_d = ins["bfc"]        # [128,2] f32
    g1_d, b1_d = ins["g1c"], ins["b1c"]   # [128,6] f32
    onesk_d = ins["onesk"]    # [128,1] bf16
    shb_d = ins["shiftbank"]  # [128,326] bf16
    y_d = outs["y"]           # [2,128,32,32] f32

    consts = ctx.enter_context(tc.tile_pool(name="consts", bufs=1))
    big = ctx.enter_context(tc.tile_pool(name="big", bufs=1))
    prodp = ctx.enter_context(tc.tile_pool(name="prodp", bufs=3))
    small = ctx.enter_context(tc.tile_pool(name="small", bufs=1))
    psA = ctx.enter_context(tc.tile_pool(name="psA", bufs=1, space="PSUM"))
    psB = ctx.enter_context(tc.tile_pool(name="psB", bufs=1, space="PSUM"))
    drp = ctx.enter_context(tc.tile_pool(name="drp", bufs=2, space="DRAM"))
    fdp = ctx.enter_context(tc.tile_pool(name="fdp", bufs=3, space="DRAM"))
    psC = ctx.enter_context(tc.tile_pool(name="psC", bufs=2, space="PSUM"))
    imgp = ctx.enter_context(tc.tile_pool(name="imgp", bufs=1, space="DRAM"))

    def psa():
        return psA.tile([128, 3, 512], F32, tag="a", name="psa_t")

    def psb():
        return psB.tile([128, 3, 512], F32, tag="b", name="psb_t")

    def load(pool, name, shape, dt, src, tag=None):
        t = pool.tile(shape, dt, tag=tag or name, name=name)
        nc.sync.dma_start(out=t, in_=src)
        return t

    # ---- input x first (padded, bf16, channel-major) so LN1 starts early ----
    xpb = big.tile([128, NCH, AW], BF16, tag="xu", name="xpb")
    for c in range(NCH):
        nc.sync.dma_start(out=xpb[:, c, 0:A], in_=xp_d[c])
    nc.vector.memset(xpb[:, :, A:AW], 0.0)
    onesk_t = load(consts, "onesk", [128, 1], BF16, onesk_d)
    bqkv_t = load(small, "bqkv", [128, 18], F32, bqkv_d)
    bproj_t = load(small, "bproj", [128, NCH], F32, bproj_d)
    bfc_t = load(small, "bfc", [128, 2], F32, bfc_d)
    g1_t = load(small, "g1c", [128, NCH], F32, g1_d)
    b1_t = load(small, "b1c", [128, NCH], F32, b1_d)


    # ---- remaining constants (overlap with LN1 compute) ----
    wq_t = consts.tile([128, NCH, 2304], BF16, tag="wq", name="wq_t")
    for lo, hi in ((0, 768), (768, 1536), (1536, 2304)):
        for c in range(NCH):
            nc.gpsimd.dma_start(out=wq_t[:, c, lo:hi], in_=wq_d[c][:, lo:hi])
    wp_t = consts.tile([128, NCH, 768], BF16, tag="wp", name="wp_t")
    wf_t = consts.tile([128, NCH, 256], BF16, tag="wf", name="wf_t")
    for c in range(NCH):
        nc.gpsimd.dma_start(out=wp_t[:, c, :], in_=wp_d[c])
        nc.gpsimd.dma_start(out=wf_t[:, c, :], in_=wf_d[c])
    shb_t = load(consts, "shiftbank", [128, 326], BF16, shb_d)
    ident_t = load(consts, "ident", [128, 128], BF16, ins["ident"])
    w2s_t = load(consts, "w2s", [1, 256], BF16, ins["w2s"])
    brow_t = load(consts, "brow", [1, 256], BF16, ins["brow"])

    # =================== LayerNorm 1 (stats over channels via PE) ============
    sqx = big.tile([128, NCH, A], BF16, tag="sq", name="sqx")
    for c in range(NCH):
        nc.scalar.activation(sqx[:, c, :], xpb[:, c, 0:A], AF.Square)

    stat1 = psa()   # sum x   [1, a]
    stat2 = psb()   # sum x^2 [1, a]
    for s, (off, w) in enumerate(SEG_A):
        for c in range(NCH):
            nc.tensor.matmul(stat1[0:1, s, 0:w], onesk_t,
                             xpb[:, c, off:off + w],
                             start=(c == 0), stop=(c == NCH - 1))
        for c in range(NCH):
            nc.tensor.matmul(stat2[0:1, s, 0:w], onesk_t,
                             sqx[:, c, off:off + w],
                             start=(c == 0), stop=(c == NCH - 1))

    def ln_smalls(stat1, stat2, width, segs, tagpfx):
        ta = small.tile([1, width], F32, tag="lnta", name=tagpfx + "ta")
        xs = small.tile([1, width], F32, tag="lnxs", name=tagpfx + "xs")
        sq = small.tile([1, width], F32, tag="lnsv", name=tagpfx + "sv")
        rstd = small.tile([1, width], BF16, tag="lnrs", name=tagpfx + "rs")
        nmur = small.tile([1, width], BF16, tag="lnnm", name=tagpfx + "nm")
        eps_t = small.tile([1, 1], F32, tag="lnep", name=tagpfx + "ep")
        nc.vector.memset(eps_t, EPS)
        s1s = small.tile([1, width], F32, tag="lns1", name="lns1")
        s1v = _ps_ragged(stat1, segs)
        s2v = _ps_ragged(stat2, segs)
        s1sv = _ragged(s1s, segs)
        tav = _ragged(ta, segs)
        xsv = _ragged(xs, segs)
        for i in range(3):
            nc.vector.tensor_copy(s1sv[i], s1v[i][0:1])
            nc.vector.tensor_tensor(tav[i], s1sv[i], s1sv[i], ALU.mult)
            nc.vector.scalar_tensor_tensor(xsv[i], tav[i], 1.0 / C, s2v[i][0:1],
                                           ALU.mult, ALU.subtract)
        nc.scalar.activation(sq, xs, AF.Sqrt, bias=eps_t, scale=-1.0 / C)
        nc.vector.reciprocal(rstd, sq)
        for i in range(3):
            nm = _ragged(nmur, segs)[i]
            rs = _ragged(rstd, segs)[i]
            nc.vector.scalar_tensor_tensor(nm, s1sv[i], -1.0 / C, rs,
                                           ALU.mult, ALU.mult)
        return rstd, nmur

    l1ta = small.tile([1, A], F32, tag="lnta", name="l1ta")
    l1xs = small.tile([1, A], F32, tag="lnxs", name="l1xs")
    l1sv = small.tile([1, A], F32, tag="lnsv", name="l1sv")
    rstd1 = small.tile([1, A], BF16, tag="lnrs", name="l1rs")
    nmur1 = small.tile([1, A], BF16, tag="lnnm", name="l1nm")
    eps1 = small.tile([1, 1], F32, tag="lnep", name="l1ep")
    nc.vector.memset(eps1, EPS)
    s1s1 = small.tile([1, A], F32, tag="lns1", name="lns1")
    rrep1 = small.tile([128, A], BF16, tag="lnrr", name="rrep1")
    nrep1 = small.tile([128, A], BF16, tag="lnnr", name="nrep1")
    rscr1 = drp.tile([1, A], BF16, tag="scr", name="rscr1")
    nscr1 = drp.tile([1, A], BF16, tag="scr", name="nscr1")
    for i, (off, w) in enumerate(SEG_A):
        sl = slice(off, off + w)
        nc.vector.tensor_copy(s1s1[:, sl], stat1[0:1, i, 0:w])
        nc.vector.tensor_tensor(l1ta[:, sl], s1s1[:, sl], s1s1[:, sl], ALU.mult)
        nc.vector.scalar_tensor_tensor(l1xs[:, sl], l1ta[:, sl], 1.0 / C,
                                       stat2[0:1, i, 0:w],
                                       ALU.mult, ALU.subtract)
        nc.scalar.activation(l1sv[:, sl], l1xs[:, sl], AF.Sqrt, bias=eps1,
                             scale=-1.0 / C)
        nc.vector.reciprocal(rstd1[:, sl], l1sv[:, sl])
        nc.vector.scalar_tensor_tensor(nmur1[:, sl], s1s1[:, sl], -1.0 / C,
                                       rstd1[:, sl], ALU.mult, ALU.mult)
        nc.sync.dma_start(out=rscr1[:, sl], in_=rstd1[:, sl])
        nc.sync.dma_start(out=nscr1[:, sl], in_=nmur1[:, sl])
        nc.sync.dma_start(out=rrep1[:, sl],
                          in_=rscr1[:, sl].to_broadcast([128, w]))
        nc.sync.dma_start(out=nrep1[:, sl],
                          in_=nscr1[:, sl].to_broadcast([128, w]))

    ln_b = big.tile([128, NCH, A], BF16, tag="ln", name="ln_b")
    for i, (off, w) in enumerate(SEG_A):
        for c in range(NCH):
            t1 = prodp.tile([128, 1158], BF16, tag="pr", name="t1")
            nc.vector.tensor_tensor(t1[:, 0:w], xpb[:, c, off:off + w],
                                    rrep1[:, off:off + w], ALU.mult)
            nc.vector.tensor_tensor(t1[:, 0:w], t1[:, 0:w],
                                    nrep1[:, off:off + w], ALU.add)
            nc.scalar.activation(ln_b[:, c, off:off + w], t1[:, 0:w],
                                 AF.Identity, bias=b1_t[:, c:c + 1],
                                 scale=g1_t[:, c:c + 1])

    # box filter (residual t_mean, x9): emitted inside the scores loop below
    t9 = big.tile([128, NCH, NW], BF16, tag="t9", name="t9")

    def emit_t9_chunk(c):
        tr = prodp.tile([128, 1158], BF16, tag="pr", name="tr")
        nc.vector.tensor_tensor(tr, xpb[:, c, 0:1158], xpb[:, c, 1:1159], ALU.add)
        nc.vector.tensor_tensor(tr, tr, xpb[:, c, 2:1160], ALU.add)
        nc.vector.tensor_tensor(t9[:, c, :], tr[:, 0:NW], tr[:, 34:34 + NW], ALU.add)
        nc.vector.tensor_tensor(t9[:, c, :], t9[:, c, :], tr[:, 68:68 + NW], ALU.add)

    # =================== qk projection (v handled transposed below) ==========
    qp = big.tile([128, NCH, A], BF16, tag="qo", name="qp")
    kp = big.tile([128, NCH, KW], BF16, tag="kp", name="kp")
    nc.vector.memset(kp[:, :, 0:70], 0.0)
    nc.vector.memset(kp[:, :, 70 + A:KW], 0.0)

    for gi, g in enumerate(list(range(6)) + list(range(6, 12))):
        ps = psa() if gi % 2 == 0 else psb()
        for s, (off, w) in enumerate(SEG_A):
            for c in range(NCH):
                nc.tensor.matmul(ps[:, s, 0:w],
                                 wq_t[:, c, 128 * g:128 * (g + 1)],
                                 ln_b[:, c, off:off + w],
                                 start=(c == 0), stop=(c == NCH - 1))
        if g < 6:
            dst = qp[:, g, 0:A]
        else:
            dst = kp[:, g - 6, 70:70 + A]
        pv = _ps_ragged(ps, SEG_A)
        dv = _ragged(dst, SEG_A)
        for i in range(3):
            if i == 1:
                nc.vector.tensor_scalar_add(dv[i], pv[i], bqkv_t[:, g:g + 1])
            else:
                nc.scalar.activation(dv[i], pv[i], AF.Identity,
                                     bias=bqkv_t[:, g:g + 1], scale=1.0)

    # ======== banded scores + exp + skew-extract + softmax + P, per tile =====
    # fsk[a-part, h, e] = exp(scale * q(a).k(a+e-70)); then
    # C1[y] = F[y] + F[y+1] + F[y+2]; C2[z] = C1[z] + C1[z+34] + C1[z+68]
    # G_i[a] = C2[70 - 34*ir - ic]; R = 1/G; PZ_i[a,(j,h)] = F(e(i,j)) * R_i
    skp = ctx.enter_context(tc.tile_pool(name="skp", bufs=2))
    skp1 = ctx.enter_context(tc.tile_pool(name="skp1", bufs=1))
    pzb = big.tile([128, 10, 9, 72], BF16, tag="pz", name="pzb")
    vT = big.tile([128, 10, C], BF16, tag="vp", name="vT")
    for c in range(NCH):
        emit_t9_chunk(c)
    for t, (a0, wa) in enumerate(AT):
        bw = wa + 140
        # transposed v for this a-tile: vT[a, c] (interleaved with the band
        # matmuls so the PE fills the exp-paced pipeline)
        psv = psa() if t % 2 == 0 else psb()
        for s in range(2):
            for c in range(NCH):
                nc.tensor.matmul(psv[0:wa, s, 0:384],
                                 ln_b[:, c, a0:a0 + wa],
                                 wq_t[:, c, 1536 + 384 * s:1536 + 384 * (s + 1)],
                                 start=(c == 0), stop=(c == NCH - 1),
                                 skip_group_check=True)
        nc.vector.tensor_copy(vT[0:wa, t, :].rearrange("p (s w) -> p s w", s=2),
                               psv[0:wa, 0:2, 0:384])
        fab = skp.tile([128, HEADS, 268], BF16, tag="fab", name="fab")
        fd = fdp.tile([128, FDW], BF16, tag="fd", name="fd")
        for gidx, (h0, nh) in enumerate(HGROUPS):
            ps = psa() if (t * 3 + gidx) % 2 == 0 else psb()
            for hh in range(nh):
                h = h0 + hh
                pieces = HEAD_PIECES[h]
                for pi, (g, p0, p1) in enumerate(pieces):
                    nc.tensor.matmul(ps[0:wa, hh, 0:bw],
                                     qp[p0:p1, g, a0:a0 + wa],
                                     kp[p0:p1, g, a0:a0 + bw],
                                     start=(pi == 0), stop=(pi == len(pieces) - 1),
                                     skip_group_check=True,
                                     tile_position=(p0, 0))
            # exp on the whole head-group band
            nc.scalar.activation(fab[0:wa, h0:h0 + nh, 0:bw],
                                 ps[0:wa, 0:nh, 0:bw], AF.Exp, scale=SCALE)
        # one DRAM roundtrip per tile: write all heads, skewed read back
        nc.sync.dma_start(
            out=fd[0:wa, :].rearrange("p (h w) -> p h w", h=HEADS),
            in_=fab[0:wa, :, :])
        fsk = skp.tile([128, HEADS, 144], BF16, tag="fsk", name="fsk")
        src = AP(fd.tensor, fd.offset, [[FDW + 1, wa], [268, HEADS], [1, 141]])
        nc.sync.dma_start(out=fsk[0:wa, :, 0:141], in_=src)

        c1 = skp1.tile([128, HEADS, 139], BF16, tag="c1", name="c1")
        nc.vector.tensor_tensor(c1[0:wa], fsk[0:wa, :, 0:139],
                                fsk[0:wa, :, 1:140], ALU.add)
        nc.vector.tensor_tensor(c1[0:wa], c1[0:wa], fsk[0:wa, :, 2:141], ALU.add)
        c2 = skp1.tile([128, HEADS, 72], BF16, tag="c2", name="c2")
        nc.vector.tensor_tensor(c2[0:wa, :, 0:71], c1[0:wa, :, 0:71],
                                c1[0:wa, :, 34:105], ALU.add)
        nc.vector.tensor_tensor(c2[0:wa, :, 0:71], c2[0:wa, :, 0:71],
                                c1[0:wa, :, 68:139], ALU.add)
        c2r = skp1.tile([128, HEADS, 72], BF16, tag="c2r", name="c2r")
        nc.vector.reciprocal(c2r[0:wa, :, 0:71], c2[0:wa, :, 0:71])

        for i, (ir, ic) in enumerate(KI_LIST):
            base = 70 - 34 * ir - ic
            in0 = AP(fsk.tensor, fsk.offset + base,
                     [[HEADS * 144, wa], [34, 3], [1, 3], [144, HEADS]])
            in1 = AP(c2r.tensor, c2r.offset + base,
                     [[HEADS * 72, wa], [0, 3], [0, 3], [72, HEADS]])
            out = AP(pzb.tensor, pzb.offset + (t * 9 + i) * 72,
                     [[10 * 9 * 72, wa], [24, 3], [8, 3], [1, HEADS]])
            nc.vector.tensor_tensor(out, in0, in1, ALU.mult)

    # =================== W via shift-diagonal matmuls ========================
    # W[n,(j,h)] = sum_i PZ_i[n + s_i, (j,h)]; scattered into the DRAM W-band
    # image right away.
    imgz = imgp.tile([IMT], BF16, tag="img", name="imgz")
    zd = imgp.tile([1, 536], BF16, tag="zd", name="zd")
    zt = small.tile([1, 536], BF16, tag="zt", name="zt")
    nc.vector.memset(zt, 0.0)
    nc.gpsimd.dma_start(out=zd, in_=zt)
    nc.gpsimd.dma_start(
        out=AP(imgz.tensor, imgz.offset, [[536, IMT // 536], [1, 536]]),
        in_=AP(zd.tensor, zd.offset, [[0, IMT // 536], [1, 536]]))

    wlb = big.tile([128, 9, 72], BF16, tag="wl", name="wlb")
    for tn, (n0, wn) in enumerate(NT):
        psw = psC.tile([128, 512], F32, tag="c", name="psw")
        mms = []
        for i, si in enumerate(S_LIST):
            for chunk in (0, 1):
                at = tn + chunk
                if at >= len(AT):
                    continue
                off = (128 + si) if chunk == 0 else si
                wa_at = AT[at][1]
                mms.append((i, si, chunk, at, off, wa_at))
        for mi, (i, si, chunk, at, off, wa_at) in enumerate(mms):
            rhs = AP(pzb.tensor, pzb.offset + (at * 9 + i) * 72,
                     [[10 * 9 * 72, wa_at], [1, 72]])
            nc.tensor.matmul(psw[0:wn, 0:72],
                             shb_t[0:wa_at, off:off + wn],
                             rhs,
                             start=(mi == 0), stop=(mi == len(mms) - 1),
                             skip_group_check=True)
        nc.scalar.activation(wlb[0:wn, tn, :], psw[0:wn, 0:72], AF.Copy,
                             scale=1.0 / 9.0)
        # scatter W values into the band image: cell (n + s_j, 70 - s_j, h);
        # DMA APs max 3 entries -> one DMA per jr (h contiguous innermost)
        for jr in range(3):
            src = AP(wlb.tensor, wlb.offset + tn * 72 + 24 * jr,
                     [[9 * 72, wn], [8, 3], [1, HEADS]])
            dst = AP(imgz.tensor,
                     imgz.offset + n0 * 2144 + (34 * 267 * jr + 70) * 8,
                     [[2144, wn], [267 * 8, 3], [1, HEADS]])
            nc.gpsimd.dma_start(out=dst, in_=src)

    # =================== o_mean via banded W matmuls, fused proj =============
    # o[c, n] = sum_a vT[a, c] * Wband_h(c)[a, n]
    o_b = big.tile([128, NCH, NW], BF16, tag="qo", name="o_b")
    u_b = big.tile([128, NCH, NW], BF16, tag="xu", name="u_b")
    stat1s = small.tile([1, NW], BF16, tag="lns1", name="stat1s")
    stat2s = small.tile([1, NW], BF16, tag="lnnm", name="stat2s")
    for tn, (n0, wn) in enumerate(NT):
        a0, wa = AT[tn]
        a1, wa1 = AT[tn + 1]
        # skewed reads: wb[ch, p, d', h] = Wband_h[a0+p, n = a0 - 70 + d']
        # n-tile tn reads rows [n0, n0+128+wa1): lo serves tn <= 3, hi tn >= 4
        wb = big.tile([128, 2, 198, HEADS], BF16,
                      tag=("sq" if tn % 2 == 0 else "kp"), name="wb")
        src0 = AP(imgz.tensor, imgz.offset + a0 * 2144,
                  [[2136, wa], [1, 198 * HEADS]])
        nc.sync.dma_start(
            out=wb[0:wa, 0, :, :].rearrange("p d h -> p (d h)"), in_=src0)
        wn1 = wn - 58
        src1 = AP(imgz.tensor, imgz.offset + a1 * 2144,
                  [[2136, wa1], [1, 198 * HEADS]])
        nc.sync.dma_start(
            out=wb[0:wa1, 1, :, :].rearrange("p d h -> p (d h)"), in_=src1)
        ps = psa() if tn % 2 == 0 else psb()
        for cch in range(NCH):
            slot, soff = cch // 4, 128 * (cch % 4)
            segs = CHUNK_SEGS[cch]
            for si_, (p0, p1, h) in enumerate(segs):
                # chunk0: n-cols [0, wn) at d' = 70 + col; chunk1: [58, wn)
                rhs0 = AP(wb.tensor, wb.offset + 70 * HEADS + h,
                          [[2 * 198 * HEADS, wa], [HEADS, wn]])
                nc.tensor.matmul(ps[p0:p1, slot, soff:soff + wn],
                                 vT[0:wa, tn, 128 * cch + p0:128 * cch + p1],
                                 rhs0,
                                 start=True, stop=False,
                                 skip_group_check=True,
                                 tile_position=(0, p0))
                rhs1 = AP(wb.tensor, wb.offset + 198 * HEADS + h,
                          [[2 * 198 * HEADS, wa1], [HEADS, wn1]])
                nc.tensor.matmul(ps[p0:p1, slot, soff + 58:soff + wn],
                                 vT[0:wa1, tn + 1, 128 * cch + p0:128 * cch + p1],
                                 rhs1,
                                 start=False, stop=True,
                                 skip_group_check=True,
                                 tile_position=(0, p0))
        for cch in range(NCH):
            slot, soff = cch // 4, 128 * (cch % 4)
            nc.scalar.activation(o_b[:, cch, n0:n0 + wn],
                                 ps[:, slot, soff:soff + wn],
                                 AF.Identity, bias=bqkv_t[:, 12 + cch:13 + cch],
                                 scale=1.0)
        # pipelined proj + residual for this n-tile
        for g in range(NCH):
            pp = psC.tile([128, 256], F32, tag="c", name="pp")
            for c in range(NCH):
                nc.tensor.matmul(pp[:, 0:wn],
                                 wp_t[:, c, 128 * g:128 * (g + 1)],
                                 o_b[:, c, n0:n0 + wn],
                                 start=(c == 0), stop=(c == NCH - 1))
            nc.vector.scalar_tensor_tensor(u_b[:, g, n0:n0 + wn],
                                           t9[:, g, n0:n0 + wn], 1.0 / 9.0,
                                           pp[:, 0:wn], ALU.mult, ALU.add)
            nc.vector.tensor_scalar_add(u_b[:, g, n0:n0 + wn],
                                        u_b[:, g, n0:n0 + wn],
                                        bproj_t[:, g:g + 1])

    # =================== LN2 stats (bulk) ===================================
    sq2 = big.tile([128, NCH, NW], BF16, tag="sq", name="sq2")
    for c in range(NCH):
        nc.vector.tensor_tensor(sq2[:, c, :], u_b[:, c, :], u_b[:, c, :],
                                ALU.mult)
    stat1b = psa()
    stat2b = psb()
    for sg, (off, w) in enumerate(SEG_N):
        for c in range(NCH):
            nc.tensor.matmul(stat1b[0:1, sg, 0:w], onesk_t,
                             u_b[:, c, off:off + w],
                             start=(c == 0), stop=(c == NCH - 1))
        for c in range(NCH):
            nc.tensor.matmul(stat2b[0:1, sg, 0:w], onesk_t,
                             sq2[:, c, off:off + w],
                             start=(c == 0), stop=(c == NCH - 1))
        nc.scalar.activation(stat1s[0:1, off:off + w], stat1b[0:1, sg, 0:w],
                             AF.Copy)
        nc.scalar.activation(stat2s[0:1, off:off + w], stat2b[0:1, sg, 0:w],
                             AF.Copy)

    # ============ folded LN2 + fc (transposed, scale at the Relu) ===========
    # y[n, o] = Relu(rstd[n] * (sum_c wf2[c,o] u[c,n] + negmu[n] W2S[o]
    #                           + sqv[n] B[o]))
    l2ta = small.tile([1, NW], F32, tag="lnta", name="l2ta")
    l2xs = small.tile([1, NW], F32, tag="lnxs", name="l2xs")
    l2sq = small.tile([1, NW], F32, tag="lnsv", name="l2sq")
    rstd2 = small.tile([1, NW], BF16, tag="lnrs", name="rstd2")
    sqv2 = small.tile([1, NW], BF16, tag="lnnr", name="sqv2")
    negmu2 = small.tile([1, NW], BF16, tag="lnrr", name="negmu2")
    eps2 = small.tile([1, 1], F32, tag="lnep", name="l2ep")
    nc.vector.memset(eps2, EPS)
    nc.vector.tensor_tensor(l2ta, stat1s, stat1s, ALU.mult)
    nc.vector.scalar_tensor_tensor(l2xs, l2ta, 1.0 / C, stat2s,
                                   ALU.mult, ALU.subtract)
    # sqv = sqrt(var + eps); rstd = 1/sqv; negmu = -mu
    nc.scalar.activation(l2sq, l2xs, AF.Sqrt, bias=eps2, scale=-1.0 / C)
    nc.vector.tensor_copy(sqv2, l2sq)
    nc.vector.reciprocal(rstd2, l2sq)
    nc.vector.tensor_scalar_mul(negmu2, stat1s, -1.0 / C)
    # rstd transposed to [n-partition, tile] via DRAM roundtrip
    rscr2 = drp.tile([1, 1152], BF16, tag="scr", name="rscr2")
    nc.sync.dma_start(out=rscr2[:, 0:NW], in_=rstd2)
    nc.sync.dma_start(out=rscr2[:, NW:1152], in_=rstd2[:, 0:64])
    rstdTb = small.tile([128, 9], BF16, tag="rstdTb", name="rstdTb")
    nc.sync.dma_start(out=rstdTb,
                      in_=AP(rscr2.tensor, rscr2.offset, [[1, 128], [128, 9]]))
    rstdT = small.tile([128, 9], F32, tag="rstdT", name="rstdT")
    nc.vector.tensor_copy(rstdT, rstdTb)

    ybuf = big.tile([128, 2, NW], F32, tag="kp", name="ybuf")
    for tn, (n0, wn) in enumerate(NT):
        psf = psa() if tn % 2 == 0 else psb()
        pf = psf[:, 0, :]
        for c in range(NCH):
            nc.tensor.matmul(pf[0:wn, 0:256], u_b[:, c, n0:n0 + wn],
                             wf_t[:, c, :],
                             start=(c == 0), stop=False,
                             skip_group_check=True)
        nc.tensor.matmul(pf[0:wn, 0:256], negmu2[0:1, n0:n0 + wn], w2s_t,
                         start=False, stop=False, skip_group_check=True)
        nc.tensor.matmul(pf[0:wn, 0:256], sqv2[0:1, n0:n0 + wn], brow_t,
                         start=False, stop=True, skip_group_check=True)
        yt = skp1.tile([128, 256], BF16, tag="yt", name="yt")
        nc.scalar.activation(yt[0:wn, :], pf[0:wn, 0:256], AF.Relu,
                             scale=rstdT[0:wn, tn:tn + 1])
        for g in range(2):
            pyt = psC.tile([128, 256], F32, tag="c", name="pyt").bitcast(BF16)
            nc.tensor.transpose(pyt[0:128, 0:wn],
                                yt[0:wn, 128 * g:128 * (g + 1)],
                                ident_t[0:wn, 0:wn])
            nc.scalar.activation(ybuf[:, g, n0:n0 + wn], pyt[0:128, 0:wn],
                                 AF.Copy)
    for g in range(2):
        src = ybuf[:, g, :].rearrange("p (r c) -> p r c", c=34)[:, :, 0:32]
        nc.sync.dma_start(out=y_d[g], in_=src)


# ============================ host-side wrapper =============================

def _build_sels():
    bf = ml_dtypes.bfloat16
    onesk = np.ones((128, 1), np.float32)
    # shiftbank[p, c] = 1 iff p == c - 128 (c in [0, 326))
    shiftbank = np.zeros((128, 326), np.float32)
    for cc in range(326):
        p = cc - 128
        if 0 <= p < 128:
            shiftbank[p, cc] = 1.0
    out = dict(onesk=onesk, shiftbank=shiftbank,
               ident=np.eye(128, dtype=np.float32))
    return {k: v.astype(bf) for k, v in out.items()}


@functools.lru_cache(maxsize=1)
def _build_module():
    nc = bacc.Bacc("TRN2", target_bir_lowering=False, debug=False)
    ins = {}

    def din(name, shape, dt):
        ins[name] = nc.dram_tensor(name, shape, dt, kind="ExternalInput").ap()

    din("xp", [NCH, 128, A], BF16)
    din("wqkv", [NCH, 128, 2304], BF16)
    din("wproj", [NCH, 128, 768], BF16)
    din("wfc", [NCH, 128, 256], BF16)
    din("bqkv", [128, 18], F32)
    din("bproj", [128, NCH], F32)
    din("bfc", [128, 2], F32)
    din("g1c", [128, NCH], F32)
    din("b1c", [128, NCH], F32)
    din("w2s", [1, 256], BF16)
    din("ident", [128, 128], BF16)
    din("brow", [1, 256], BF16)
    din("onesk", [128, 1], BF16)
    din("shiftbank", [128, 326], BF16)
    outs = {"y": nc.dram_tensor("y", [2, 128, 32, 32], F32,
                                kind="ExternalOutput").ap()}

    from contextlib import ExitStack
    with tile.TileContext(nc) as tc:
        with ExitStack() as ctx:
            with nc.allow_low_precision(reason="bf16 kernel by design"):
                emit_kernel(ctx, tc, ins, outs)
    nc.compile()
    return nc


def kernel(x, w_qkv, b_qkv, w_proj, b_proj, g1, beta1, g2, beta2, w_fc, b_fc,
           _run_kwargs=None):
    bf = ml_dtypes.bfloat16
    x = np.asarray(x, np.float32)
    B = x.shape[0]
    assert x.shape == (8, C, 32, 32)

    sels = _build_sels()
    shared = dict(
        wqkv=np.ascontiguousarray(
            np.asarray(w_qkv, np.float32).reshape(NCH, 128, 2304)).astype(bf),
        wproj=np.ascontiguousarray(
            np.asarray(w_proj, np.float32).reshape(NCH, 128, 768)).astype(bf),
        wfc=np.ascontiguousarray(
            (np.asarray(w_fc, np.float32)
             * np.asarray(g2, np.float32)[:, None]).reshape(
                NCH, 128, 256)).astype(bf),
        w2s=(np.asarray(w_fc, np.float32)
             * np.asarray(g2, np.float32)[:, None]).sum(0)[None, :].astype(bf),
        brow=(np.asarray(w_fc, np.float32).T @ np.asarray(beta2, np.float32)
              + np.asarray(b_fc, np.float32))[None, :].astype(bf),
        bqkv=np.ascontiguousarray(
            np.asarray(b_qkv, np.float32).reshape(18, 128).T),
        bproj=np.ascontiguousarray(
            np.asarray(b_proj, np.float32).reshape(NCH, 128).T),
        bfc=np.ascontiguousarray(np.asarray(b_fc, np.float32).reshape(2, 128).T),
        g1c=np.ascontiguousarray(np.asarray(g1, np.float32).reshape(NCH, 128).T),
        b1c=np.ascontiguousarray(np.asarray(beta1, np.float32).reshape(NCH, 128).T),

        **sels,
    )
    in_maps = []
    for b in range(B):
        xpad = np.pad(x[b], ((0, 0), (1, 1), (1, 1)), mode="edge")
        xp = np.ascontiguousarray(xpad.reshape(NCH, 128, A)).astype(bf)
        in_maps.append(dict(xp=xp, **shared))

    nc = _build_module()
    res = run_bass_kernel_spmd(nc, in_maps, core_ids=list(range(8)),
                               **(_run_kwargs or {}))
    outs = []
    for b in range(B):
        y = np.asarray(res.results[b]["y"], np.float32)  # [2,128,32,32]
        outs.append(y.reshape(256, 32, 32))
    out = np.stack(outs).astype(np.float32)
    if _run_kwargs is not None:
        kernel.last_result = res
    return out


# revision 5
# speedup vs baseline: 1.0160x; 1.0160x over previous
"""Trainium2 Bass kernel for nn_AttnBlock (sparse 3x3-window attention).

Restructuring (~2x vs the previous kernel, TimelineSim ~208us/core):
  - Scores: banded q.k^T matmuls on the PE per a-tile of 128 pixels (band of
    268 absolute positions, per-head partition-subrange contraction), instead
    of 150 DVE product ops + PE selection-matmul reduction.
  - exp() on the whole band on ACT; the 25 displacement maps F_e[a] are then
    extracted with a skewed DMA read through a DRAM roundtrip (diagonal access
    patterns are expressible on flat DRAM, not on SBUF).
  - Softmax denominators: 3x3 box sums along the displacement axis (DVE+Pool);
    P = F * 1/G; window column-sums W[n,(j,h)] assembled with constant
    shift-diagonal matmuls (one wide diagonal "shiftbank" constant).
  - o_mean: W is scattered into a zero-filled DRAM band image (h-innermost so
    the scatter has 16B runs), read back as skewed [a, n, h] tiles, and o =
    vT^T @ Wband runs as banded PE matmuls against a transposed v (produced
    directly by stationary-swapped qkv matmuls).  proj is pipelined per n-tile.
  - LN2 is algebraically folded into a transposed fc: y^T = Relu(rstd[n] *
    (u^T @ (wfc*g2) + negmu[n]*colsum + sqrtvar[n]*bias_row)), with rstd as a
    per-partition ACT scale; the y tiles are PE-transposed back to [o, n].

Sharding: data-parallel over batch B=8 -> one batch per NeuronCore.
"""

import functools
import numpy as np
import ml_dtypes

import concourse.bass as bass
import concourse.mybir as mybir
import concourse.tile as tile
from concourse import bacc
from concourse.bass_utils import run_bass_kernel_spmd

F32 = mybir.dt.float32
BF16 = mybir.dt.bfloat16
AF = mybir.ActivationFunctionType
ALU = mybir.AluOpType
AP = bass.AP

C = 768
NCH = 6          # channel chunks of 128
G = 34           # padded grid side
A = G * G        # 1156 padded pixels
AW = 1160        # padded-pixel width with 4 pad cols
NW = 1088        # window-grid width = 32*34 (rows 0..31, cols 0..33)
KW = 1300        # k map width with +-70 margins (content at 70)
HEADS = 8
HD = 96
SCALE = HD ** -0.5
EPS = 1e-5

# segments over the a-grid (1156) and n-grid (1088); PSUM tile is [P, 3, 512]
SEG_A = [(0, 386), (386, 386), (772, 384)]
SEG_N = [(0, 384), (384, 384), (768, 320)]

KI_LIST = [(r, c) for r in range(3) for c in range(3)]             # 9
S_LIST = [34 * r + c for (r, c) in KI_LIST]                        # window offsets

# a-tiles and n-tiles of 128
AT = [(128 * t, 128) for t in range(9)] + [(1152, 4)]              # 10 tiles
NT = [(128 * t, 128) for t in range(8)] + [(1024, 64)]             # 9 tiles

# head h -> list of (chunk, p0, p1) pieces covering d-range [96h, 96h+96).
# PE tile_position rules: size<=32 -> base in {0,32,64,96}; size<=64 -> {0,64};
# else base 0.  Split pieces starting at 32 so each is legal.
def _head_pieces(h):
    lo, hi = 96 * h, 96 * h + 96
    out = []
    g0, g1 = lo // 128, (hi - 1) // 128
    for g in range(g0, g1 + 1):
        p0 = max(lo - 128 * g, 0)
        p1 = min(hi - 128 * g, 128)
        if p0 == 32 and p1 > 64:
            out.append((g, 32, 64))
            out.append((g, 64, p1))
        else:
            out.append((g, p0, p1))
    return out

HEAD_PIECES = [_head_pieces(h) for h in range(HEADS)]
# head groups per psum tile: 3 + 3 + 2
HGROUPS = [(0, 3), (3, 3), (6, 2)]
FDW = 2144       # dram band pitch: 8 heads x 268

# W-band image, h-interleaved: cell (a, d', h) at flat (a*268 + d')*8 + h;
# content = W[n = a - 70 + d', j: s_j = 70 - d', h] for d' in {70 - s}, else 0
IMR = 268                # image row pitch (in cells)
IMT = 1160 * IMR * HEADS


# c-chunk -> list of (p0, p1, h) out-partition segments with legal tile pos
def _chunk_segs(cch):
    lo = 128 * cch
    bounds = sorted({lo, lo + 128} |
                    {96 * h for h in range(1, 8) if lo < 96 * h < lo + 128})
    segs = []
    for b0, b1 in zip(bounds[:-1], bounds[1:]):
        p0, p1 = b0 - lo, b1 - lo
        h = b0 // 96
        if p0 == 32 and p1 - p0 > 32:
            segs.append((32, 64, h))
            segs.append((64, p1, h))
        else:
            segs.append((p0, p1, h))
    return segs


CHUNK_SEGS = [_chunk_segs(c) for c in range(NCH)]


def _ragged(ap_flat, segs):
    return [ap_flat[:, o:o + w] for (o, w) in segs]


def _ps_ragged(ps, segs):
    return [ps[:, s, 0:w] for s, (o, w) in enumerate(segs)]


def emit_kernel(ctx, tc, ins, outs):
    nc = tc.nc
    xp_d = ins["xp"]          # [6,128,1156] bf16
    wq_d = ins["wqkv"]        # [6,128,2304] bf16
    wp_d = ins["wproj"]       # [6,128,768] bf16
    wf_d = ins["wfc"]         # [6,128,256] bf16
    bqkv_d = ins["bqkv"]      # [128,18] f32
    bproj_d = ins["bproj"]    # [128,6] f32
    bfc_d = ins["bfc"]        # [128,2] f32
    g1_d, b1_d = ins["g1c"], ins["b1c"]   # [128,6] f32
    onesk_d = ins["onesk"]    # [128,1] bf16
    shb_d = ins["shiftbank"]  # [128,326] bf16
    y_d = outs["y"]           # [2,128,32,32] f32

    consts = ctx.enter_context(tc.tile_pool(name="consts", bufs=1))
    big = ctx.enter_context(tc.tile_pool(name="big", bufs=1))
    prodp = ctx.enter_context(tc.tile_pool(name="prodp", bufs=3))
    small = ctx.enter_context(tc.tile_pool(name="small", bufs=1))
    psA = ctx.enter_context(tc.tile_pool(name="psA", bufs=1, space="PSUM"))
    psB = ctx.enter_context(tc.tile_pool(name="psB", bufs=1, space="PSUM"))
    drp = ctx.enter_context(tc.tile_pool(name="drp", bufs=2, space="DRAM"))
    fdp = ctx.enter_context(tc.tile_pool(name="fdp", bufs=3, space="DRAM"))
    psC = ctx.enter_context(tc.tile_pool(name="psC", bufs=2, space="PSUM"))
    imgp = ctx.enter_context(tc.tile_pool(name="imgp", bufs=1, space="DRAM"))

    def psa():
        return psA.tile([128, 3, 512], F32, tag="a", name="psa_t")

    def psb():
        return psB.tile([128, 3, 512], F32, tag="b", name="psb_t")

    def load(pool, name, shape, dt, src, tag=None):
        t = pool.tile(shape, dt, tag=tag or name, name=name)
        nc.sync.dma_start(out=t, in_=src)
        return t

    # ---- input x first (padded, bf16, channel-major) so LN1 starts early ----
    xpb = big.tile([128, NCH, AW], BF16, tag="xu", name="xpb")
    for c in range(NCH):
        nc.sync.dma_start(out=xpb[:, c, 0:A], in_=xp_d[c])
    nc.vector.memset(xpb[:, :, A:AW], 0.0)
    onesk_t = load(consts, "onesk", [128, 1], BF16, onesk_d)
    bqkv_t = load(small, "bqkv", [128, 18], F32, bqkv_d)
    bproj_t = load(small, "bproj", [128, NCH], F32, bproj_d)
    bfc_t = load(small, "bfc", [128, 2], F32, bfc_d)
    g1_t = load(small, "g1c", [128, NCH], F32, g1_d)
    b1_t = load(small, "b1c", [128, NCH], F32, b1_d)


    # ---- remaining constants (overlap with LN1 compute) ----
    wq_t = consts.tile([128, NCH, 2304], BF16, tag="wq", name="wq_t")
    for lo, hi in ((0, 768), (768, 1536), (1536, 2304)):
        for c in range(NCH):
            nc.gpsimd.dma_start(out=wq_t[:, c, lo:hi], in_=wq_d[c][:, lo:hi])
    wp_t = consts.tile([128, NCH, 768], BF16, tag="wp", name="wp_t")
    wf_t = consts.tile([128, NCH, 256], BF16, tag="wf", name="wf_t")
    for c in range(NCH):
        nc.gpsimd.dma_start(out=wp_t[:, c, :], in_=wp_d[c])
        nc.gpsimd.dma_start(out=wf_t[:, c, :], in_=wf_d[c])
    shb_t = load(consts, "shiftbank", [128, 326], BF16, shb_d)
    ident_t = load(consts, "ident", [128, 128], BF16, ins["ident"])
    w2s_t = load(consts, "w2s", [1, 256], BF16, ins["w2s"])
    brow_t = load(consts, "brow", [1, 256], BF16, ins["brow"])

    # =================== LayerNorm 1 (stats over channels via PE) ============
    sqx = big.tile([128, NCH, A], BF16, tag="sq", name="sqx")
    for c in range(NCH):
        nc.scalar.activation(sqx[:, c, :], xpb[:, c, 0:A], AF.Square)

    stat1 = psa()   # sum x   [1, a]
    stat2 = psb()   # sum x^2 [1, a]
    for s, (off, w) in enumerate(SEG_A):
        for c in range(NCH):
            nc.tensor.matmul(stat1[0:1, s, 0:w], onesk_t,
                             xpb[:, c, off:off + w],
                             start=(c == 0), stop=(c == NCH - 1))
        for c in range(NCH):
            nc.tensor.matmul(stat2[0:1, s, 0:w], onesk_t,
                             sqx[:, c, off:off + w],
                             start=(c == 0), stop=(c == NCH - 1))

    def ln_smalls(stat1, stat2, width, segs, tagpfx):
        ta = small.tile([1, width], F32, tag="lnta", name=tagpfx + "ta")
        xs = small.tile([1, width], F32, tag="lnxs", name=tagpfx + "xs")
        sq = small.tile([1, width], F32, tag="lnsv", name=tagpfx + "sv")
        rstd = small.tile([1, width], BF16, tag="lnrs", name=tagpfx + "rs")
        nmur = small.tile([1, width], BF16, tag="lnnm", name=tagpfx + "nm")
        eps_t = small.tile([1, 1], F32, tag="lnep", name=tagpfx + "ep")
        nc.vector.memset(eps_t, EPS)
        s1s = small.tile([1, width], F32, tag="lns1", name="lns1")
        s1v = _ps_ragged(stat1, segs)
        s2v = _ps_ragged(stat2, segs)
        s1sv = _ragged(s1s, segs)
        tav = _ragged(ta, segs)
        xsv = _ragged(xs, segs)
        for i in range(3):
            nc.vector.tensor_copy(s1sv[i], s1v[i][0:1])
            nc.vector.tensor_tensor(tav[i], s1sv[i], s1sv[i], ALU.mult)
            nc.vector.scalar_tensor_tensor(xsv[i], tav[i], 1.0 / C, s2v[i][0:1],
                                           ALU.mult, ALU.subtract)
        nc.scalar.activation(sq, xs, AF.Sqrt, bias=eps_t, scale=-1.0 / C)
        nc.vector.reciprocal(rstd, sq)
        for i in range(3):
            nm = _ragged(nmur, segs)[i]
            rs = _ragged(rstd, segs)[i]
            nc.vector.scalar_tensor_tensor(nm, s1sv[i], -1.0 / C, rs,
                                           ALU.mult, ALU.mult)
        return rstd, nmur

    l1ta = small.tile([1, A], F32, tag="lnta", name="l1ta")
    l1xs = small.tile([1, A], F32, tag="lnxs", name="l1xs")
    l1sv = small.tile([1, A], F32, tag="lnsv", name="l1sv")
    rstd1 = small.tile([1, A], BF16, tag="lnrs", name="l1rs")
    nmur1 = small.tile([1, A], BF16, tag="lnnm", name="l1nm")
    eps1 = small.tile([1, 1], F32, tag="lnep", name="l1ep")
    nc.vector.memset(eps1, EPS)
    s1s1 = small.tile([1, A], F32, tag="lns1", name="lns1")
    rrep1 = small.tile([128, A], BF16, tag="lnrr", name="rrep1")
    nrep1 = small.tile([128, A], BF16, tag="lnnr", name="nrep1")
    rscr1 = drp.tile([1, A], BF16, tag="scr", name="rscr1")
    nscr1 = drp.tile([1, A], BF16, tag="scr", name="nscr1")
    for i, (off, w) in enumerate(SEG_A):
        sl = slice(off, off + w)
        nc.vector.tensor_copy(s1s1[:, sl], stat1[0:1, i, 0:w])
        nc.vector.tensor_tensor(l1ta[:, sl], s1s1[:, sl], s1s1[:, sl], ALU.mult)
        nc.vector.scalar_tensor_tensor(l1xs[:, sl], l1ta[:, sl], 1.0 / C,
                                       stat2[0:1, i, 0:w],
                                       ALU.mult, ALU.subtract)
        nc.scalar.activation(l1sv[:, sl], l1xs[:, sl], AF.Sqrt, bias=eps1,
                             scale=-1.0 / C)
        nc.vector.reciprocal(rstd1[:, sl], l1sv[:, sl])
        nc.vector.scalar_tensor_tensor(nmur1[:, sl], s1s1[:, sl], -1.0 / C,
                                       rstd1[:, sl], ALU.mult, ALU.mult)
        nc.sync.dma_start(out=rscr1[:, sl], in_=rstd1[:, sl])
        nc.sync.dma_start(out=nscr1[:, sl], in_=nmur1[:, sl])
        nc.sync.dma_start(out=rrep1[:, sl],
                          in_=rscr1[:, sl].to_broadcast([128, w]))
        nc.sync.dma_start(out=nrep1[:, sl],
                          in_=nscr1[:, sl].to_broadcast([128, w]))

    ln_b = big.tile([128, NCH, A], BF16, tag="ln", name="ln_b")
    for i, (off, w) in enumerate(SEG_A):
        for c in range(NCH):
            t1 = prodp.tile([128, 1158], BF16, tag="pr", name="t1")
            nc.vector.tensor_tensor(t1[:, 0:w], xpb[:, c, off:off + w],
                                    rrep1[:, off:off + w], ALU.mult)
            nc.vector.tensor_tensor(t1[:, 0:w], t1[:, 0:w],
                                    nrep1[:, off:off + w], ALU.add)
            nc.scalar.activation(ln_b[:, c, off:off + w], t1[:, 0:w],
                                 AF.Identity, bias=b1_t[:, c:c + 1],
                                 scale=g1_t[:, c:c + 1])

    # box filter (residual t_mean, x9): emitted inside the scores loop below
    t9 = big.tile([128, NCH, NW], BF16, tag="t9", name="t9")

    def emit_t9_chunk(c):
        tr = prodp.tile([128, 1158], BF16, tag="pr", name="tr")
        nc.vector.tensor_tensor(tr, xpb[:, c, 0:1158], xpb[:, c, 1:1159], ALU.add)
        nc.vector.tensor_tensor(tr, tr, xpb[:, c, 2:1160], ALU.add)
        nc.vector.tensor_tensor(t9[:, c, :], tr[:, 0:NW], tr[:, 34:34 + NW], ALU.add)
        nc.vector.tensor_tensor(t9[:, c, :], t9[:, c, :], tr[:, 68:68 + NW], ALU.add)

    # =================== qk projection (v handled transposed below) ==========
    qp = big.tile([128, NCH, A], BF16, tag="qo", name="qp")
    kp = big.tile([128, NCH, KW], BF16, tag="kp", name="kp")
    nc.vector.memset(kp[:, :, 0:70], 0.0)
    nc.vector.memset(kp[:, :, 70 + A:KW], 0.0)

    for gi, g in enumerate(list(range(6)) + list(range(6, 12))):
        ps = psa() if gi % 2 == 0 else psb()
        for s, (off, w) in enumerate(SEG_A):
            for c in range(NCH):
                nc.tensor.matmul(ps[:, s, 0:w],
                                 wq_t[:, c, 128 * g:128 * (g + 1)],
                                 ln_b[:, c, off:off + w],
                                 start=(c == 0), stop=(c == NCH - 1))
        if g < 6:
            dst = qp[:, g, 0:A]
        else:
            dst = kp[:, g - 6, 70:70 + A]
        pv = _ps_ragged(ps, SEG_A)
        dv = _ragged(dst, SEG_A)
        for i in range(3):
            if i == 1:
                nc.vector.tensor_scalar_add(dv[i], pv[i], bqkv_t[:, g:g + 1])
            else:
                nc.scalar.activation(dv[i], pv[i], AF.Identity,
                                     bias=bqkv_t[:, g:g + 1], scale=1.0)

    # ======== banded scores + exp + skew-extract + softmax + P, per tile =====
    # fsk[a-part, h, e] = exp(scale * q(a).k(a+e-70)); then
    # C1[y] = F[y] + F[y+1] + F[y+2]; C2[z] = C1[z] + C1[z+34] + C1[z+68]
    # G_i[a] = C2[70 - 34*ir - ic]; R = 1/G; PZ_i[a,(j,h)] = F(e(i,j)) * R_i
    skp = ctx.enter_context(tc.tile_pool(name="skp", bufs=2))
    skp1 = ctx.enter_context(tc.tile_pool(name="skp1", bufs=1))
    pzb = big.tile([128, 10, 9, 72], BF16, tag="pz", name="pzb")
    vT = big.tile([128, 10, C], BF16, tag="vp", name="vT")
    for c in range(NCH):
        emit_t9_chunk(c)
    for t, (a0, wa) in enumerate(AT):
        bw = wa + 140
        # transposed v for this a-tile: vT[a, c] (interleaved with the band
        # matmuls so the PE fills the exp-paced pipeline)
        psv = psa() if t % 2 == 0 else psb()
        for s in range(2):
            for c in range(NCH):
                nc.tensor.matmul(psv[0:wa, s, 0:384],
                                 ln_b[:, c, a0:a0 + wa],
                                 wq_t[:, c, 1536 + 384 * s:1536 + 384 * (s + 1)],
                                 start=(c == 0), stop=(c == NCH - 1),
                                 skip_group_check=True)
        nc.vector.tensor_copy(vT[0:wa, t, :].rearrange("p (s w) -> p s w", s=2),
                               psv[0:wa, 0:2, 0:384])
        fab = skp.tile([128, HEADS, 268], BF16, tag="fab", name="fab")
        fd = fdp.tile([128, FDW], BF16, tag="fd", name="fd")
        for gidx, (h0, nh) in enumerate(HGROUPS):
            ps = psa() if (t * 3 + gidx) % 2 == 0 else psb()
            for hh in range(nh):
                h = h0 + hh
                pieces = HEAD_PIECES[h]
                for pi, (g, p0, p1) in enumerate(pieces):
                    nc.tensor.matmul(ps[0:wa, hh, 0:bw],
                                     qp[p0:p1, g, a0:a0 + wa],
                                     kp[p0:p1, g, a0:a0 + bw],
                                     start=(pi == 0), stop=(pi == len(pieces) - 1),
                                     skip_group_check=True,
                                     tile_position=(p0, 0))
            # exp on the whole head-group band
            nc.scalar.activation(fab[0:wa, h0:h0 + nh, 0:bw],
                                 ps[0:wa, 0:nh, 0:bw], AF.Exp, scale=SCALE)
        # one DRAM roundtrip per tile: write all heads, skewed read back
        nc.sync.dma_start(
            out=fd[0:wa, :].rearrange("p (h w) -> p h w", h=HEADS),
            in_=fab[0:wa, :, :])
        fsk = skp.tile([128, HEADS, 144], BF16, tag="fsk", name="fsk")
        src = AP(fd.tensor, fd.offset, [[FDW + 1, wa], [268, HEADS], [1, 141]])
        nc.sync.dma_start(out=fsk[0:wa, :, 0:141], in_=src)

        c1 = skp1.tile([128, HEADS, 139], BF16, tag="c1", name="c1")
        nc.vector.tensor_tensor(c1[0:wa], fsk[0:wa, :, 0:139],
                                fsk[0:wa, :, 1:140], ALU.add)
        nc.vector.tensor_tensor(c1[0:wa], c1[0:wa], fsk[0:wa, :, 2:141], ALU.add)
        c2 = skp1.tile([128, HEADS, 72], BF16, tag="c2", name="c2")
        nc.vector.tensor_tensor(c2[0:wa, :, 0:71], c1[0:wa, :, 0:71],
                                c1[0:wa, :, 34:105], ALU.add)
        nc.vector.tensor_tensor(c2[0:wa, :, 0:71], c2[0:wa, :, 0:71],
                                c1[0:wa, :, 68:139], ALU.add)
        c2r = skp1.tile([128, HEADS, 72], BF16, tag="c2r", name="c2r")
        nc.vector.reciprocal(c2r[0:wa, :, 0:71], c2[0:wa, :, 0:71])

        for i, (ir, ic) in enumerate(KI_LIST):
            base = 70 - 34 * ir - ic
            in0 = AP(fsk.tensor, fsk.offset + base,
                     [[HEADS * 144, wa], [34, 3], [1, 3], [144, HEADS]])
            in1 = AP(c2r.tensor, c2r.offset + base,
                     [[HEADS * 72, wa], [0, 3], [0, 3], [72, HEADS]])
            out = AP(pzb.tensor, pzb.offset + (t * 9 + i) * 72,
                     [[10 * 9 * 72, wa], [24, 3], [8, 3], [1, HEADS]])
            nc.vector.tensor_tensor(out, in0, in1, ALU.mult)

    # =================== W via shift-diagonal matmuls ========================
    # W[n,(j,h)] = sum_i PZ_i[n + s_i, (j,h)]; scattered into the DRAM W-band
    # image right away.
    imgz = imgp.tile([IMT], BF16, tag="img", name="imgz")
    zd = imgp.tile([1, 536], BF16, tag="zd", name="zd")
    zt = small.tile([1, 536], BF16, tag="zt", name="zt")
    nc.vector.memset(zt, 0.0)
    nc.gpsimd.dma_start(out=zd, in_=zt)
    nc.gpsimd.dma_start(
        out=AP(imgz.tensor, imgz.offset, [[536, IMT // 536], [1, 536]]),
        in_=AP(zd.tensor, zd.offset, [[0, IMT // 536], [1, 536]]))

    wlb = big.tile([128, 9, 72], BF16, tag="wl", name="wlb")
    for tn, (n0, wn) in enumerate(NT):
        psw = psC.tile([128, 512], F32, tag="c", name="psw")
        mms = []
        for i, si in enumerate(S_LIST):
            for chunk in (0, 1):
                at = tn + chunk
                if at >= len(AT):
                    continue
                off = (128 + si) if chunk == 0 else si
                wa_at = AT[at][1]
                mms.append((i, si, chunk, at, off, wa_at))
        for mi, (i, si, chunk, at, off, wa_at) in enumerate(mms):
            rhs = AP(pzb.tensor, pzb.offset + (at * 9 + i) * 72,
                     [[10 * 9 * 72, wa_at], [1, 72]])
            nc.tensor.matmul(psw[0:wn, 0:72],
                             shb_t[0:wa_at, off:off + wn],
                             rhs,
                             start=(mi == 0), stop=(mi == len(mms) - 1),
                             skip_group_check=True)
        nc.scalar.activation(wlb[0:wn, tn, :], psw[0:wn, 0:72], AF.Copy,
                             scale=1.0 / 9.0)
        # scatter W values into the band image: cell (n + s_j, 70 - s_j, h);
        # DMA APs max 3 entries -> one DMA per jr (h contiguous innermost)
        for jr in range(3):
            src = AP(wlb.tensor, wlb.offset + tn * 72 + 24 * jr,
                     [[9 * 72, wn], [8, 3], [1, HEADS]])
            dst = AP(imgz.tensor,
                     imgz.offset + n0 * 2144 + (34 * 267 * jr + 70) * 8,
                     [[2144, wn], [267 * 8, 3], [1, HEADS]])
            nc.gpsimd.dma_start(out=dst, in_=src)

    # =================== o_mean via banded W matmuls, fused proj =============
    # o[c, n] = sum_a vT[a, c] * Wband_h(c)[a, n]
    o_b = big.tile([128, NCH, NW], BF16, tag="qo", name="o_b")
    u_b = big.tile([128, NCH, NW], BF16, tag="xu", name="u_b")
    stat1s = small.tile([1, NW], BF16, tag="lns1", name="stat1s")
    stat2s = small.tile([1, NW], BF16, tag="lnnm", name="stat2s")
    for tn, (n0, wn) in enumerate(NT):
        a0, wa = AT[tn]
        a1, wa1 = AT[tn + 1]
        # skewed reads: wb[ch, p, d', h] = Wband_h[a0+p, n = a0 - 70 + d']
        # n-tile tn reads rows [n0, n0+128+wa1): lo serves tn <= 3, hi tn >= 4
        wb = big.tile([128, 2, 198, HEADS], BF16,
                      tag=("sq" if tn % 2 == 0 else "kp"), name="wb")
        src0 = AP(imgz.tensor, imgz.offset + a0 * 2144,
                  [[2136, wa], [1, 198 * HEADS]])
        nc.sync.dma_start(
            out=wb[0:wa, 0, :, :].rearrange("p d h -> p (d h)"), in_=src0)
        wn1 = wn - 58
        src1 = AP(imgz.tensor, imgz.offset + a1 * 2144,
                  [[2136, wa1], [1, 198 * HEADS]])
        nc.sync.dma_start(
            out=wb[0:wa1, 1, :, :].rearrange("p d h -> p (d h)"), in_=src1)
        ps = psa() if tn % 2 == 0 else psb()
        for cch in range(NCH):
            slot, soff = cch // 4, 128 * (cch % 4)
            segs = CHUNK_SEGS[cch]
            for si_, (p0, p1, h) in enumerate(segs):
                # chunk0: n-cols [0, wn) at d' = 70 + col; chunk1: [58, wn)
                rhs0 = AP(wb.tensor, wb.offset + 70 * HEADS + h,
                          [[2 * 198 * HEADS, wa], [HEADS, wn]])
                nc.tensor.matmul(ps[p0:p1, slot, soff:soff + wn],
                                 vT[0:wa, tn, 128 * cch + p0:128 * cch + p1],
                                 rhs0,
                                 start=True, stop=False,
                                 skip_group_check=True,
                                 tile_position=(0, p0))
                rhs1 = AP(wb.tensor, wb.offset + 198 * HEADS + h,
                          [[2 * 198 * HEADS, wa1], [HEADS, wn1]])
                nc.tensor.matmul(ps[p0:p1, slot, soff + 58:soff + wn],
                                 vT[0:wa1, tn + 1, 128 * cch + p0:128 * cch + p1],
                                 rhs1,
                                 start=False, stop=True,
                                 skip_group_check=True,
                                 tile_position=(0, p0))
        for cch in range(NCH):
            slot, soff = cch // 4, 128 * (cch % 4)
            nc.scalar.activation(o_b[:, cch, n0:n0 + wn],
                                 ps[:, slot, soff:soff + wn],
                                 AF.Identity, bias=bqkv_t[:, 12 + cch:13 + cch],
                                 scale=1.0)
        # pipelined proj + residual for this n-tile
        for g in range(NCH):
            pp = psC.tile([128, 256], F32, tag="c", name="pp")
            for c in range(NCH):
                nc.tensor.matmul(pp[:, 0:wn],
                                 wp_t[:, c, 128 * g:128 * (g + 1)],
                                 o_b[:, c, n0:n0 + wn],
                                 start=(c == 0), stop=(c == NCH - 1))
            nc.vector.scalar_tensor_tensor(u_b[:, g, n0:n0 + wn],
                                           t9[:, g, n0:n0 + wn], 1.0 / 9.0,
                                           pp[:, 0:wn], ALU.mult, ALU.add)
            nc.vector.tensor_scalar_add(u_b[:, g, n0:n0 + wn],
                                        u_b[:, g, n0:n0 + wn],
                                        bproj_t[:, g:g + 1])

    # =================== LN2 stats (bulk) ===================================
    sq2 = big.tile([128, NCH, NW], BF16, tag="sq", name="sq2")
    for c in range(NCH):
        nc.vector.tensor_tensor(sq2[:, c, :], u_b[:, c, :], u_b[:, c, :],
                                ALU.mult)
    stat1b = psa()
    stat2b = psb()
    for sg, (off, w) in enumerate(SEG_N):
        for c in range(NCH):
            nc.tensor.matmul(stat1b[0:1, sg, 0:w], onesk_t,
                             u_b[:, c, off:off + w],
                             start=(c == 0), stop=(c == NCH - 1))
        for c in range(NCH):
            nc.tensor.matmul(stat2b[0:1, sg, 0:w], onesk_t,
                             sq2[:, c, off:off + w],
                             start=(c == 0), stop=(c == NCH - 1))
        nc.scalar.activation(stat1s[0:1, off:off + w], stat1b[0:1, sg, 0:w],
                             AF.Copy)
        nc.scalar.activation(stat2s[0:1, off:off + w], stat2b[0:1, sg, 0:w],
                             AF.Copy)

    # ============ folded LN2 + fc (transposed, scale at the Relu) ===========
    # y[n, o] = Relu(rstd[n] * (sum_c wf2[c,o] u[c,n] + negmu[n] W2S[o]
    #                           + sqv[n] B[o]))
    l2ta = small.tile([1, NW], F32, tag="lnta", name="l2ta")
    l2xs = small.tile([1, NW], F32, tag="lnxs", name="l2xs")
    l2sq = small.tile([1, NW], F32, tag="lnsv", name="l2sq")
    rstd2 = small.tile([1, NW], BF16, tag="lnrs", name="rstd2")
    sqv2 = small.tile([1, NW], BF16, tag="lnnr", name="sqv2")
    negmu2 = small.tile([1, NW], BF16, tag="lnrr", name="negmu2")
    eps2 = small.tile([1, 1], F32, tag="lnep", name="l2ep")
    nc.vector.memset(eps2, EPS)
    nc.vector.tensor_tensor(l2ta, stat1s, stat1s, ALU.mult)
    nc.vector.scalar_tensor_tensor(l2xs, l2ta, 1.0 / C, stat2s,
                                   ALU.mult, ALU.subtract)
    # sqv = sqrt(var + eps); rstd = 1/sqv; negmu = -mu
    nc.scalar.activation(l2sq, l2xs, AF.Sqrt, bias=eps2, scale=-1.0 / C)
    nc.vector.tensor_copy(sqv2, l2sq)
    nc.vector.reciprocal(rstd2, l2sq)
    nc.vector.tensor_scalar_mul(negmu2, stat1s, -1.0 / C)
    # rstd transposed to [n-partition, tile] via DRAM roundtrip
    rscr2 = drp.tile([1, 1152], BF16, tag="scr", name="rscr2")
    nc.sync.dma_start(out=rscr2[:, 0:NW], in_=rstd2)
    nc.sync.dma_start(out=rscr2[:, NW:1152], in_=rstd2[:, 0:64])
    rstdTb = small.tile([128, 9], BF16, tag="rstdTb", name="rstdTb")
    nc.sync.dma_start(out=rstdTb,
                      in_=AP(rscr2.tensor, rscr2.offset, [[1, 128], [128, 9]]))
    rstdT = small.tile([128, 9], F32, tag="rstdT", name="rstdT")
    nc.vector.tensor_copy(rstdT, rstdTb)

    ybuf = big.tile([128, 2, NW], F32, tag="kp", name="ybuf")
    for tn, (n0, wn) in enumerate(NT):
        psf = psa() if tn % 2 == 0 else psb()
        pf = psf[:, 0, :]
        for c in range(NCH):
            nc.tensor.matmul(pf[0:wn, 0:256], u_b[:, c, n0:n0 + wn],
                             wf_t[:, c, :],
                             start=(c == 0), stop=False,
                             skip_group_check=True)
        nc.tensor.matmul(pf[0:wn, 0:256], negmu2[0:1, n0:n0 + wn], w2s_t,
                         start=False, stop=False, skip_group_check=True)
        nc.tensor.matmul(pf[0:wn, 0:256], sqv2[0:1, n0:n0 + wn], brow_t,
                         start=False, stop=True, skip_group_check=True)
        yt = skp1.tile([128, 256], BF16, tag="yt", name="yt")
        nc.scalar.activation(yt[0:wn, :], pf[0:wn, 0:256], AF.Relu,
                             scale=rstdT[0:wn, tn:tn + 1])
        for g in range(2):
            pyt = psC.tile([128, 256], F32, tag="c", name="pyt").bitcast(BF16)
            nc.tensor.transpose(pyt[0:128, 0:wn],
                                yt[0:wn, 128 * g:128 * (g + 1)],
                                ident_t[0:wn, 0:wn])
            nc.scalar.activation(ybuf[:, g, n0:n0 + wn], pyt[0:128, 0:wn],
                                 AF.Copy)
    for g in range(2):
        src = ybuf[:, g, :].rearrange("p (r c) -> p r c", c=34)[:, :, 0:32]
        nc.sync.dma_start(out=y_d[g], in_=src)


# ============================ host-side wrapper =============================

def _build_sels():
    bf = ml_dtypes.bfloat16
    onesk = np.ones((128, 1), np.float32)
    # shiftbank[p, c] = 1 iff p == c - 128 (c in [0, 326))
    shiftbank = np.zeros((128, 326), np.float32)
    for cc in range(326):
        p = cc - 128
        if 0 <= p < 128:
            shiftbank[p, cc] = 1.0
    out = dict(onesk=onesk, shiftbank=shiftbank,
               ident=np.eye(128, dtype=np.float32))
    return {k: v.astype(bf) for k, v in out.items()}


@functools.lru_cache(maxsize=1)
def _build_module():
    nc = bacc.Bacc("TRN2", target_bir_lowering=False, debug=False)
    ins = {}

    def din(name, shape, dt):
        ins[name] = nc.dram_tensor(name, shape, dt, kind="ExternalInput").ap()

    din("xp", [NCH, 128, A], BF16)
    din("wqkv", [NCH, 128, 2304], BF16)
    din("wproj", [NCH, 128, 768], BF16)
    din("wfc", [NCH, 128, 256], BF16)
    din("bqkv", [128, 18], F32)
    din("bproj", [128, NCH], F32)
    din("bfc", [128, 2], F32)
    din("g1c", [128, NCH], F32)
    din("b1c", [128, NCH], F32)
    din("w2s", [1, 256], BF16)
    din("ident", [128, 128], BF16)
    din("brow", [1, 256], BF16)
    din("onesk", [128, 1], BF16)
    din("shiftbank", [128, 326], BF16)
    outs = {"y": nc.dram_tensor("y", [2, 128, 32, 32], F32,
                                kind="ExternalOutput").ap()}

    from contextlib import ExitStack
    with tile.TileContext(nc) as tc:
        with ExitStack() as ctx:
            with nc.allow_low_precision(reason="bf16 kernel by design"):
                emit_kernel(ctx, tc, ins, outs)
    nc.compile()
    return nc


def kernel(x, w_qkv, b_qkv, w_proj, b_proj, g1, beta1, g2, beta2, w_fc, b_fc,
           _run_kwargs=None):
    bf = ml_dtypes.bfloat16
    x = np.asarray(x, np.float32)
    B = x.shape[0]
    assert x.shape == (8, C, 32, 32)

    sels = _build_sels()
    shared = dict(
        wqkv=np.ascontiguousarray(
            np.asarray(w_qkv, np.float32).reshape(NCH, 128, 2304)).astype(bf),
        wproj=np.ascontiguousarray(
            np.asarray(w_proj, np.float32).reshape(NCH, 128, 768)).astype(bf),
        wfc=np.ascontiguousarray(
            (np.asarray(w_fc, np.float32)
             * np.asarray(g2, np.float32)[:, None]).reshape(
                NCH, 128, 256)).astype(bf),
        w2s=(np.asarray(w_fc, np.float32)
             * np.asarray(g2, np.float32)[:, None]).sum(0)[None, :].astype(bf),
        brow=(np.asarray(w_fc, np.float32).T @ np.asarray(beta2, np.float32)
              + np.asarray(b_fc, np.float32))[None, :].astype(bf),
        bqkv=np.ascontiguousarray(
            np.asarray(b_qkv, np.float32).reshape(18, 128).T),
        bproj=np.ascontiguousarray(
            np.asarray(b_proj, np.float32).reshape(NCH, 128).T),
        bfc=np.ascontiguousarray(np.asarray(b_fc, np.float32).reshape(2, 128).T),
        g1c=np.ascontiguousarray(np.asarray(g1, np.float32).reshape(NCH, 128).T),
        b1c=np.ascontiguousarray(np.asarray(beta1, np.float32).reshape(NCH, 128).T),

        **sels,
    )
    in_maps = []
    for b in range(B):
        xpad = np.pad(x[b], ((0, 0), (1, 1), (1, 1)), mode="edge")
        xp = np.ascontiguousarray(xpad.reshape(NCH, 128, A)).astype(bf)
        in_maps.append(dict(xp=xp, **shared))

    nc = _build_module()
    res = run_bass_kernel_spmd(nc, in_maps, core_ids=list(range(8)),
                               **(_run_kwargs or {}))
    outs = []
    for b in range(B):
        y = np.asarray(res.results[b]["y"], np.float32)  # [2,128,32,32]
        outs.append(y.reshape(256, 32, 32))
    out = np.stack(outs).astype(np.float32)
    if _run_kwargs is not None:
        kernel.last_result = res
    return out
